# revision 9
# baseline (speedup 1.0000x reference)
"""Trainium2 Bass kernel v3 for nn_CustomLayerMKM: y = x @ (sum_k kron(Bk, Ak)).T + bias.

Fully stride-free dataflow. The i-block index t rides in the matmul FREE
dimension (x stored [r, b2, t, bl], b-pair bl innermost), so:
  stage 1:  MM out psum free = (b2, t, bl)  -> contiguous [128,1024] evictions
            into U_k [p=(blk,c), b2, t, bl]
  corner turn: U viewed as uint32 (b-pairs) [p, b2, t] -> DVE 32x32 block
            stream-transpose (natural APs, both contiguous) -> V_k [p=(blk,t), b2, c, bl]
  stage 2:  rhs = V_k[:, 8*bs:+8] = [128, 512] contiguous; out psum free =
            (b2, c, bl) -> contiguous y evictions -> y_ext [p, b2, c, bl]
All matmuls are N=512 with the small pattern matrices as stationary.
Data-parallel over B across 8 cores; bf16 compute; y bf16 (host upcasts).

Index map (per factor k, Ak m x m, Bk f x f, m*f=4096, G=128/m blocks/i-block):
  o-tile c holds outputs {o : o mod 32 == c} at partition p, o = rho_inv[p]*32+c.
"""

from contextlib import ExitStack

import numpy as np

P = 128
B_FULL, I_DIM, O_DIM = 4096, 4096, 4096
N_CORES = 8
B_SHARD = B_FULL // N_CORES          # 512
NB2 = B_SHARD // 2                   # 256 b-pairs
TB = I_DIM // P                      # 32 i-blocks
NC_TILES = 32
N_FAC = 3
NBS = 32                             # b-slices of 16 rows (8 b-pairs)
NG = 8                               # groups of 4 b-slices

_BITREV2 = [0, 2, 1, 3]


def _rho():
    rho = np.zeros(128, np.int64)
    for ob2 in range(128):
        rho[ob2] = ((ob2 & 1) << 6) | (((ob2 >> 1) & 1) << 5) | (ob2 >> 2)
    rho_inv = np.zeros(128, np.int64)
    rho_inv[rho] = np.arange(128)
    return rho, rho_inv


def build_nc(debug_dump=False):
    import concourse.bass as bass
    import concourse.mybir as mybir
    import concourse.tile as tile
    from concourse import bacc

    BF16 = mybir.dt.bfloat16
    F32 = mybir.dt.float32
    U32 = mybir.dt.uint32

    nc = bacc.Bacc("TRN2", target_bir_lowering=False, debug=False,
                   num_devices=N_CORES)

    # x: [r, b2, t, bl]  (host pre-transposed)
    xT_ext = nc.dram_tensor("xT", [P, NB2, TB, 2], BF16,
                            kind="ExternalInput").ap()
    # all six 128x128 patterns packed in one tensor -> one fast DMA
    pats_ext = nc.dram_tensor("pats", [P, 2 * N_FAC, P], BF16,
                              kind="ExternalInput").ap()
    # y: [p, b2, c, bl]
    y_ext = nc.dram_tensor("y", [P, NB2, NC_TILES, 2], BF16,
                           kind="ExternalOutput").ap()

    with tile.TileContext(nc) as tc, ExitStack() as ctx:
        const = ctx.enter_context(tc.tile_pool(name="const", bufs=1))
        ps1 = ctx.enter_context(tc.tile_pool(name="ps1", bufs=1, space="PSUM"))
        ps2 = ctx.enter_context(tc.tile_pool(name="ps2", bufs=1, space="PSUM"))
        data = ctx.enter_context(tc.tile_pool(name="data", bufs=1))
        ypool = ctx.enter_context(tc.tile_pool(name="ypool", bufs=4))

        pats_sb = const.tile([P, 2 * N_FAC, P], BF16, tag="pats")
        nc.sync.dma_start(pats_sb[:], pats_ext[:])
        patA = [pats_sb[:, k, :] for k in range(N_FAC)]
        patB = [pats_sb[:, N_FAC + k, :] for k in range(N_FAC)]

        # warm the PE/HAM during the x-load lead-in with dummy matmuls
        warm = ps2.tile([P, 2, 512], F32, tag="s2", name="warm")
        for _ in range(24):
            nc.tensor.matmul(warm[:, 0, 0:128], patA[0], patA[0],
                             start=True, stop=True)

        x_sb = data.tile([P, NB2, TB, 2], BF16, tag="x")
        nc.sync.dma_start(x_sb[:, 0:8], xT_ext[:, 0:8])
        nc.sync.dma_start(x_sb[:, 8:32], xT_ext[:, 8:32])
        for ch in range(1, NG):
            sl = slice(32 * ch, 32 * ch + 32)
            nc.sync.dma_start(x_sb[:, sl], xT_ext[:, sl])

        upool = ctx.enter_context(tc.tile_pool(name="upool", bufs=2))
        vpool = ctx.enter_context(tc.tile_pool(name="vpool", bufs=4))

        # greedy eviction-engine assignment by projected busy-time (us);
        # DVE is pre-loaded by the stream-transposes it alone can run
        load = {"dve": 0.0, "act": 0.0}

        def evict(dst, src):
            if load["act"] + 1.12 <= load["dve"] + 1.20:
                nc.scalar.copy(dst, src)
                load["act"] += 1.12
            else:
                nc.vector.tensor_copy(dst, src)
                load["dve"] += 1.20

        n_s1 = [0]

        def stage2_half(g, Vg, sl):
            yslab = ps2.tile([P, 2, 512], F32, tag="s2",
                             name=f"s2_{g}_{sl}")
            for i in range(2):
                bsl = sl * 2 + i
                for k in range(N_FAC):
                    nc.tensor.matmul(
                        yslab[:, i, :],
                        patB[k],
                        Vg[k][:, 8 * bsl:8 * bsl + 8],
                        start=(k == 0), stop=(k == N_FAC - 1))
            y_sb = ypool.tile([P, 2, 512], BF16, tag="y",
                              name=f"y_{g}_{sl}")
            evict(y_sb[:], yslab[:])
            b2lo = 16 * (g * 2 + sl)
            nc.sync.dma_start(y_ext[:, b2lo:b2lo + 16], y_sb[:])

        Vq = []          # pending V groups awaiting stage 2 (lag 2 groups)
        for g in range(NG):
            # ---- stage 1 (k-blocks); transpose right after each factor ----
            Vg = []
            for k in range(N_FAC):
                U = upool.tile([P, 32, TB, 2], BF16, tag=f"U{k}",
                               name=f"U{g}_{k}")
                for sl in range(2):
                    slab = ps1.tile([P, 2, 512], F32,
                                    tag=f"s1_{n_s1[0] % 3}",
                                    name=f"s1_{g}_{k}_{sl}")
                    n_s1[0] += 1
                    for i in range(2):
                        bs = g * 4 + sl * 2 + i
                        nc.tensor.matmul(
                            slab[:, i, :],
                            patA[k],
                            x_sb[:, 8 * bs:8 * bs + 8],
                            start=True, stop=True)
                    evict(U[:, 16 * sl:16 * sl + 16], slab[:])
                V = vpool.tile([P, 32, NC_TILES, 2], BF16, tag=f"V{k}",
                               name=f"V{g}_{k}")
                Vg.append(V)
                nc.vector.transpose(
                    V[:].bitcast(U32).squeeze(),
                    U[:].bitcast(U32).squeeze())
                load["dve"] += 1.22
                # interleave a stage-2 half of the group from 2 iterations ago
                if len(Vq) == 2 and k < 2:
                    stage2_half(g - 2, Vq[0], k)
            if len(Vq) == 2:
                Vq.pop(0)
            Vq.append(Vg)
        for j, gg in enumerate((NG - 2, NG - 1)):
            stage2_half(gg, Vq[j], 0)
            stage2_half(gg, Vq[j], 1)

    nc.compile()
    return nc


_NC_CACHE = {}


def _patterns(inputs):
    """Six 128x128 pattern matrices (f32; cast to bf16 by caller)."""
    rho, rho_inv = _rho()
    A = [np.asarray(inputs[n], np.float32) for n in ("w0a", "w1a", "w2a")]
    B = [np.asarray(inputs[n], np.float32) for n in ("w0b", "w1b", "w2b")]
    A0, A1, A2 = A
    B0, B1, B2 = B

    pa0 = np.zeros((P, P), np.float32)
    for g in range(2):
        for hh in range(2):
            pa0[g * 64:(g + 1) * 64, 64 * hh + 32 * g:64 * hh + 32 * g + 32] = \
                A0[32 * hh:32 * hh + 32, :].T
    pa1 = np.zeros((P, P), np.float32)
    for s in range(4):
        pa1[:, 32 * s:32 * s + 32] = A1[32 * _BITREV2[s]:32 * _BITREV2[s] + 32, :].T
    pa2 = np.zeros((P, P), np.float32)
    for g in range(4):
        pa2[g * 32:(g + 1) * 32, g * 32:(g + 1) * 32] = A2.T

    t_idx = np.arange(32)
    pb0 = np.zeros((P, P), np.float32)
    for hh in range(2):
        cols = np.arange(64 * hh, 64 * hh + 64)
        ob0 = rho_inv[cols] >> 1
        for g in range(2):
            pb0[64 * hh + 32 * g:64 * hh + 32 * g + 32, cols] = \
                B0[np.ix_(ob0, 2 * t_idx + g)].T
    pb1 = np.zeros((P, P), np.float32)
    for s in range(4):
        pb1[32 * s:32 * s + 32, 32 * s:32 * s + 32] = B1.T
    pb2 = np.zeros((P, P), np.float32)
    for g in range(4):
        pb2[32 * g:32 * g + 32, :] = B2[np.ix_(rho_inv, 4 * t_idx + g)].T

    return {"patA0": pa0, "patA1": pa1, "patA2": pa2,
            "patB0": pb0, "patB1": pb1, "patB2": pb2}


def prep_inputs(inputs):
    """Host prep: per-core bf16 x in [r, b2, t, bl] + pattern matrices."""
    import ml_dtypes

    bf16 = ml_dtypes.bfloat16
    x = np.asarray(inputs["input_BI"], dtype=np.float32)
    pats = _patterns(inputs)
    packed = np.stack([pats["patA0"], pats["patA1"], pats["patA2"],
                       pats["patB0"], pats["patB1"], pats["patB2"]], axis=1)
    common = {"pats": np.ascontiguousarray(packed.astype(bf16))}

    in_maps = []
    for c in range(N_CORES):
        im = dict(common)
        xs = x[c * B_SHARD:(c + 1) * B_SHARD]            # [512, 4096]
        # [b2, bl, t, r] -> [r, b2, t, bl]
        xr = xs.reshape(NB2, 2, TB, P).transpose(3, 0, 2, 1)
        im["xT"] = np.ascontiguousarray(xr.astype(bf16))
        in_maps.append(im)
    return in_maps


def unshuffle_y(y_raw):
    """[p, b2, c, bl] -> [512, 4096] f32 (no bias)."""
    _, rho_inv = _rho()
    # [p, b2, c, bl] -> [b2, bl, p, c] -> [512, 4096]
    yt = np.asarray(y_raw).astype(np.float32).transpose(1, 3, 0, 2)
    yt = yt.reshape(B_SHARD, P * NC_TILES)
    o_idx = (rho_inv[:, None] * 32 + np.arange(NC_TILES)[None, :]).ravel()
    out = np.empty((B_SHARD, O_DIM), np.float32)
    out[:, o_idx] = yt
    return out


def kernel(**inputs):
    """Full-input entry point: shards over B, runs 8-core SPMD, gathers."""
    from concourse.bass_utils import run_bass_kernel_spmd

    in_maps = prep_inputs(inputs)
    if "nc" not in _NC_CACHE:
        _NC_CACHE["nc"] = build_nc()
    res = run_bass_kernel_spmd(_NC_CACHE["nc"], in_maps,
                               core_ids=list(range(N_CORES)))
    bias = np.asarray(inputs["bias_O"], dtype=np.float32)[None, :]
    y = np.concatenate([unshuffle_y(r["y"]) for r in res.results], axis=0)
    return y + bias


# revision 10
# speedup vs baseline: 1.0353x; 1.0353x over previous
"""Trainium2 Bass kernel v3 for nn_CustomLayerMKM: y = x @ (sum_k kron(Bk, Ak)).T + bias.

Fully stride-free dataflow. The i-block index t rides in the matmul FREE
dimension (x stored [r, b2, t, bl], b-pair bl innermost), so:
  stage 1:  MM out psum free = (b2, t, bl)  -> contiguous [128,1024] evictions
            into U_k [p=(blk,c), b2, t, bl]
  corner turn: U viewed as uint32 (b-pairs) [p, b2, t] -> DVE 32x32 block
            stream-transpose (natural APs, both contiguous) -> V_k [p=(blk,t), b2, c, bl]
  stage 2:  rhs = V_k[:, 8*bs:+8] = [128, 512] contiguous; out psum free =
            (b2, c, bl) -> contiguous y evictions -> y_ext [p, b2, c, bl]
All matmuls are N=512 with the small pattern matrices as stationary.
Data-parallel over B across 8 cores; bf16 compute; y bf16 (host upcasts).

Index map (per factor k, Ak m x m, Bk f x f, m*f=4096, G=128/m blocks/i-block):
  o-tile c holds outputs {o : o mod 32 == c} at partition p, o = rho_inv[p]*32+c.
"""

from contextlib import ExitStack

import numpy as np

P = 128
B_FULL, I_DIM, O_DIM = 4096, 4096, 4096
N_CORES = 8
B_SHARD = B_FULL // N_CORES          # 512
NB2 = B_SHARD // 2                   # 256 b-pairs
TB = I_DIM // P                      # 32 i-blocks
NC_TILES = 32
N_FAC = 3
NBS = 32                             # b-slices of 16 rows (8 b-pairs)
NG = 8                               # groups of 4 b-slices

_BITREV2 = [0, 2, 1, 3]


def _rho():
    rho = np.zeros(128, np.int64)
    for ob2 in range(128):
        rho[ob2] = ((ob2 & 1) << 6) | (((ob2 >> 1) & 1) << 5) | (ob2 >> 2)
    rho_inv = np.zeros(128, np.int64)
    rho_inv[rho] = np.arange(128)
    return rho, rho_inv


def build_nc(debug_dump=False):
    import concourse.bass as bass
    import concourse.mybir as mybir
    import concourse.tile as tile
    from concourse import bacc

    BF16 = mybir.dt.bfloat16
    F32 = mybir.dt.float32
    U32 = mybir.dt.uint32

    nc = bacc.Bacc("TRN2", target_bir_lowering=False, debug=False,
                   num_devices=N_CORES)

    # x: [r, b2, t, bl]  (host pre-transposed)
    xT_ext = nc.dram_tensor("xT", [P, NB2, TB, 2], BF16,
                            kind="ExternalInput").ap()
    # all six 128x128 patterns packed in one tensor -> one fast DMA
    pats_ext = nc.dram_tensor("pats", [P, 2 * N_FAC, P], BF16,
                              kind="ExternalInput").ap()
    # y: [p, b2, c, bl]
    y_ext = nc.dram_tensor("y", [P, NB2, NC_TILES, 2], BF16,
                           kind="ExternalOutput").ap()

    with tile.TileContext(nc) as tc, ExitStack() as ctx:
        const = ctx.enter_context(tc.tile_pool(name="const", bufs=1))
        ps1 = ctx.enter_context(tc.tile_pool(name="ps1", bufs=1, space="PSUM"))
        ps2 = ctx.enter_context(tc.tile_pool(name="ps2", bufs=1, space="PSUM"))
        data = ctx.enter_context(tc.tile_pool(name="data", bufs=1))
        ypool = ctx.enter_context(tc.tile_pool(name="ypool", bufs=4))

        pats_sb = const.tile([P, 2 * N_FAC, P], BF16, tag="pats")
        nc.sync.dma_start(pats_sb[:], pats_ext[:])
        patA = [pats_sb[:, k, :] for k in range(N_FAC)]
        patB = [pats_sb[:, N_FAC + k, :] for k in range(N_FAC)]

        # warm the PE/HAM during the x-load lead-in with dummy matmuls
        warm = ps2.tile([P, 2, 512], F32, tag="s2", name="warm")
        for _ in range(24):
            nc.tensor.matmul(warm[:, 0, 0:128], patA[0], patA[0],
                             start=True, stop=True)

        x_sb = data.tile([P, NB2, TB, 2], BF16, tag="x")
        nc.sync.dma_start(x_sb[:, 0:8], xT_ext[:, 0:8])
        nc.sync.dma_start(x_sb[:, 8:32], xT_ext[:, 8:32])
        for ch in range(1, NG):
            sl = slice(32 * ch, 32 * ch + 32)
            nc.sync.dma_start(x_sb[:, sl], xT_ext[:, sl])

        upool = ctx.enter_context(tc.tile_pool(name="upool", bufs=2))
        vpool = ctx.enter_context(tc.tile_pool(name="vpool", bufs=3))

        # greedy eviction-engine assignment by projected busy-time (us);
        # DVE is pre-loaded by the stream-transposes it alone can run
        load = {"dve": 0.0, "act": 0.0}

        def evict(dst, src):
            if load["act"] + 1.12 <= load["dve"] + 1.20:
                nc.scalar.copy(dst, src)
                load["act"] += 1.12
            else:
                nc.vector.tensor_copy(dst, src)
                load["dve"] += 1.20

        n_s1 = [0]

        def stage2_half(g, Vg, sl):
            yslab = ps2.tile([P, 2, 512], F32, tag="s2",
                             name=f"s2_{g}_{sl}")
            for i in range(2):
                bsl = sl * 2 + i
                for k in range(N_FAC):
                    nc.tensor.matmul(
                        yslab[:, i, :],
                        patB[k],
                        Vg[k][:, 8 * bsl:8 * bsl + 8],
                        start=(k == 0), stop=(k == N_FAC - 1))
            y_sb = ypool.tile([P, 2, 512], BF16, tag="y",
                              name=f"y_{g}_{sl}")
            evict(y_sb[:], yslab[:])
            b2lo = 16 * (g * 2 + sl)
            nc.sync.dma_start(y_ext[:, b2lo:b2lo + 16], y_sb[:])

        Vq = []          # pending V groups awaiting stage 2 (lag 2 groups)
        for g in range(NG):
            # ---- stage 1 (k-blocks); transpose right after each factor ----
            Vg = []
            for k in range(N_FAC):
                U = upool.tile([P, 32, TB, 2], BF16, tag=f"U{k}",
                               name=f"U{g}_{k}")
                for sl in range(2):
                    slab = ps1.tile([P, 2, 512], F32,
                                    tag=f"s1_{n_s1[0] % 3}",
                                    name=f"s1_{g}_{k}_{sl}")
                    n_s1[0] += 1
                    for i in range(2):
                        bs = g * 4 + sl * 2 + i
                        nc.tensor.matmul(
                            slab[:, i, :],
                            patA[k],
                            x_sb[:, 8 * bs:8 * bs + 8],
                            start=True, stop=True)
                    evict(U[:, 16 * sl:16 * sl + 16], slab[:])
                V = vpool.tile([P, 32, NC_TILES, 2], BF16, tag=f"V{k}",
                               name=f"V{g}_{k}")
                Vg.append(V)
                nc.vector.transpose(
                    V[:].bitcast(U32).squeeze(),
                    U[:].bitcast(U32).squeeze())
                load["dve"] += 1.22
                # interleave a stage-2 half of the group from 2 iterations ago
                if len(Vq) == 2 and k < 2:
                    stage2_half(g - 2, Vq[0], k)
            if len(Vq) == 2:
                Vq.pop(0)
            Vq.append(Vg)
        for j, gg in enumerate((NG - 2, NG - 1)):
            stage2_half(gg, Vq[j], 0)
            stage2_half(gg, Vq[j], 1)

    nc.compile()
    return nc


_NC_CACHE = {}


def _patterns(inputs):
    """Six 128x128 pattern matrices (f32; cast to bf16 by caller)."""
    rho, rho_inv = _rho()
    A = [np.asarray(inputs[n], np.float32) for n in ("w0a", "w1a", "w2a")]
    B = [np.asarray(inputs[n], np.float32) for n in ("w0b", "w1b", "w2b")]
    A0, A1, A2 = A
    B0, B1, B2 = B

    pa0 = np.zeros((P, P), np.float32)
    for g in range(2):
        for hh in range(2):
            pa0[g * 64:(g + 1) * 64, 64 * hh + 32 * g:64 * hh + 32 * g + 32] = \
                A0[32 * hh:32 * hh + 32, :].T
    pa1 = np.zeros((P, P), np.float32)
    for s in range(4):
        pa1[:, 32 * s:32 * s + 32] = A1[32 * _BITREV2[s]:32 * _BITREV2[s] + 32, :].T
    pa2 = np.zeros((P, P), np.float32)
    for g in range(4):
        pa2[g * 32:(g + 1) * 32, g * 32:(g + 1) * 32] = A2.T

    t_idx = np.arange(32)
    pb0 = np.zeros((P, P), np.float32)
    for hh in range(2):
        cols = np.arange(64 * hh, 64 * hh + 64)
        ob0 = rho_inv[cols] >> 1
        for g in range(2):
            pb0[64 * hh + 32 * g:64 * hh + 32 * g + 32, cols] = \
                B0[np.ix_(ob0, 2 * t_idx + g)].T
    pb1 = np.zeros((P, P), np.float32)
    for s in range(4):
        pb1[32 * s:32 * s + 32, 32 * s:32 * s + 32] = B1.T
    pb2 = np.zeros((P, P), np.float32)
    for g in range(4):
        pb2[32 * g:32 * g + 32, :] = B2[np.ix_(rho_inv, 4 * t_idx + g)].T

    return {"patA0": pa0, "patA1": pa1, "patA2": pa2,
            "patB0": pb0, "patB1": pb1, "patB2": pb2}


def prep_inputs(inputs):
    """Host prep: per-core bf16 x in [r, b2, t, bl] + pattern matrices."""
    import ml_dtypes

    bf16 = ml_dtypes.bfloat16
    x = np.asarray(inputs["input_BI"], dtype=np.float32)
    pats = _patterns(inputs)
    packed = np.stack([pats["patA0"], pats["patA1"], pats["patA2"],
                       pats["patB0"], pats["patB1"], pats["patB2"]], axis=1)
    common = {"pats": np.ascontiguousarray(packed.astype(bf16))}

    in_maps = []
    for c in range(N_CORES):
        im = dict(common)
        xs = x[c * B_SHARD:(c + 1) * B_SHARD]            # [512, 4096]
        # [b2, bl, t, r] -> [r, b2, t, bl]
        xr = xs.reshape(NB2, 2, TB, P).transpose(3, 0, 2, 1)
        im["xT"] = np.ascontiguousarray(xr.astype(bf16))
        in_maps.append(im)
    return in_maps


def unshuffle_y(y_raw):
    """[p, b2, c, bl] -> [512, 4096] f32 (no bias)."""
    _, rho_inv = _rho()
    # [p, b2, c, bl] -> [b2, bl, p, c] -> [512, 4096]
    yt = np.asarray(y_raw).astype(np.float32).transpose(1, 3, 0, 2)
    yt = yt.reshape(B_SHARD, P * NC_TILES)
    o_idx = (rho_inv[:, None] * 32 + np.arange(NC_TILES)[None, :]).ravel()
    out = np.empty((B_SHARD, O_DIM), np.float32)
    out[:, o_idx] = yt
    return out


def kernel(**inputs):
    """Full-input entry point: shards over B, runs 8-core SPMD, gathers."""
    from concourse.bass_utils import run_bass_kernel_spmd

    in_maps = prep_inputs(inputs)
    if "nc" not in _NC_CACHE:
        _NC_CACHE["nc"] = build_nc()
    res = run_bass_kernel_spmd(_NC_CACHE["nc"], in_maps,
                               core_ids=list(range(N_CORES)))
    bias = np.asarray(inputs["bias_O"], dtype=np.float32)[None, :]
    y = np.concatenate([unshuffle_y(r["y"]) for r in res.results], axis=0)
    return y + bias
